# revision 3
# baseline (speedup 1.0000x reference)
"""Block-diagonal grouped matmul (nn_MatrixApply) on 8 TRN2 NeuronCores.

Math: out[s:s+g] = mat_i @ x[s:s+g] for 15 consecutive sample groups.
Equivalently out = BD @ x_flat with BD = blockdiag(mat_0..mat_14) (2048x2048)
and x_flat = x.reshape(2048, 512*21).

Sharding: sequence-parallel. The free dim L*A = 10752 is split into 8
contiguous chunks of 1344. Mats are replicated. No collectives; host
concatenates the slices.

Numerics: fp16 in / fp16 out, single matmul per tile, f32 PSUM accumulate.
Measured rel err ~5e-4 (tolerance 2e-2). This halves both input and output
HBM bytes vs the f32-equivalent and cuts PE work 3x vs bf16 hi/lo.

Layout: the 2048-sample dim is row-PERMUTED on the host into 16 bins of
exactly 128 rows (big groups split into 128-chunks; small groups/remainders
bin-packed to exact-128 bins — zero padding). x is pre-packed on the host
into the literal SBUF image (chunk, partition, bin*cols) so each chunk is
ONE fully contiguous DMA in and one out (vs 40 small strided DMAs). All
matmul tiles are full 128x128; a tile exists for each bin pair sharing a
group (36 tiles). Output is un-permuted on the host.

Per-core steady state: DMA 11MB/rep (~34us at the ~320GB/s/NC effective HBM
rate) overlaps PE (108 matmuls x 448 cols ~ 20us) and PSUM evacuation split
across the vector and scalar engines.
"""

import numpy as np

import concourse.bacc as bacc
import concourse.mybir as mybir
import concourse.tile as tile
from concourse import bass_utils

GROUP_SIZES = (64, 128, 256, 96, 160, 224, 192, 288, 320, 112, 80, 48, 32, 16, 32)
LENGTH = 512
ALPHABET = 21
N_SAMPLES = 2048
N_CORES = 8
FREE = LENGTH * ALPHABET            # 10752
FREE_PER_CORE = FREE // N_CORES     # 1344
P = 128
NBINS = N_SAMPLES // P              # 16
F16 = np.float16

# DMA chunking (cols per in/out DMA) and PSUM sub-chunk (cols per matmul
# accumulation pass; must divide DMA_COLS, <= 512 f32 PSUM bank).
DMA_COLS = 448
PS_COLS = 448
BUFS = 3
SPLIT_RINGS = False


def _plan():
    """Row permutation into 16 exact-128 bins + matmul tile list.

    Pieces: each group is split into 128-row chunks plus a remainder;
    full chunks become bins directly, remainders/small groups are
    bin-packed (first-fit decreasing) into exact-128 bins. Returns
    (perm, ptiles): perm[i] = global sample row at packed position i;
    ptiles = [(mb, kb)] bin pairs sharing a group (full 128x128 tiles).
    """
    starts = np.cumsum((0,) + GROUP_SIZES[:-1])
    full_bins = []
    pieces = []
    for g, (gs, s0) in enumerate(zip(GROUP_SIZES, starts)):
        o = 0
        while gs - o >= P:
            full_bins.append([(g, s0 + o, P)])
            o += P
        if gs - o:
            pieces.append((gs - o, g, s0 + o))
    packed = []
    space = []
    for sz, g, s0 in sorted(pieces, reverse=True):
        for i in range(len(packed)):
            if space[i] >= sz:
                packed[i].append((g, s0, sz))
                space[i] -= sz
                break
        else:
            packed.append([(g, s0, sz)])
            space.append(P - sz)
    assert all(s == 0 for s in space), space
    bins = full_bins + packed
    assert len(bins) == NBINS
    perm = np.concatenate(
        [np.arange(s0, s0 + sz) for b in bins for (_, s0, sz) in b]
    )
    assert len(perm) == N_SAMPLES
    bin_groups = [{g for (g, _, _) in b} for b in bins]
    ptiles = [
        (mb, kb)
        for mb in range(NBINS)
        for kb in range(NBINS)
        if bin_groups[mb] & bin_groups[kb]
    ]
    return perm, ptiles


PERM, PTILES = _plan()
N_TILES = len(PTILES)               # 36
ROW_TILES = [
    [(t, kb) for t, (mb, kb) in enumerate(PTILES) if mb == i] for i in range(NBINS)
]


def build_program(reps=1, dma_cols=DMA_COLS, ps_cols=PS_COLS, bufs=BUFS,
                  split_rings=SPLIT_RINGS):
    """Per-core Bass program. reps>1 repeats the streaming body in-NEFF
    for wall-clock differencing benchmarks."""
    assert FREE_PER_CORE % dma_cols == 0 and dma_cols % ps_cols == 0
    n_chunks = FREE_PER_CORE // dma_cols
    n_pass = dma_cols // ps_cols
    nc = bacc.Bacc("TRN2", target_bir_lowering=False, debug=False)
    f32 = mybir.dt.float32
    f16 = mybir.dt.float16
    xp_d = nc.dram_tensor("xp", (n_chunks, P, NBINS * dma_cols), f16,
                          kind="ExternalInput")
    w_d = nc.dram_tensor("wpack", (P, N_TILES * P), f16, kind="ExternalInput")
    op_d = nc.dram_tensor("out", (n_chunks, P, NBINS * dma_cols), f16,
                          kind="ExternalOutput")

    def dma_in(xt, c):
        if split_rings:
            h = NBINS * dma_cols // 2
            nc.sync.dma_start(xt[:, 0:h], xp_d.ap()[c, :, 0:h])
            nc.scalar.dma_start(xt[:, h:], xp_d.ap()[c, :, h:])
        else:
            nc.sync.dma_start(xt[:], xp_d.ap()[c])

    def dma_out(ot, c):
        if split_rings:
            h = NBINS * dma_cols // 2
            nc.scalar.dma_start(op_d.ap()[c, :, 0:h], ot[:, 0:h])
            nc.sync.dma_start(op_d.ap()[c, :, h:], ot[:, h:])
        else:
            nc.scalar.dma_start(op_d.ap()[c], ot[:])

    with tile.TileContext(nc) as tc:
        with (
            tc.tile_pool(name="wpool", bufs=1) as wpool,
            tc.tile_pool(name="xpool", bufs=bufs) as xpool,
            tc.tile_pool(name="opool", bufs=bufs) as opool,
            tc.tile_pool(name="psum", bufs=8, space="PSUM") as psum_pool,
        ):
            w_sb = wpool.tile([P, N_TILES * P], f16)
            nc.scalar.dma_start(w_sb[:], w_d.ap())
            for _rep in range(reps):
                for c in range(n_chunks):
                    xt = xpool.tile([P, NBINS * dma_cols], f16, tag="x")
                    dma_in(xt, c)
                    ot = opool.tile([P, NBINS * dma_cols], f16, tag="o")
                    for pc in range(n_pass):
                        for mb in range(NBINS):
                            ps = psum_pool.tile([P, ps_cols], f32, tag="ps")
                            mms = ROW_TILES[mb]
                            for k, (t, kb) in enumerate(mms):
                                nc.tensor.matmul(
                                    ps[:],
                                    w_sb[:, t * P:(t + 1) * P],
                                    xt[:, kb * dma_cols + pc * ps_cols:
                                       kb * dma_cols + (pc + 1) * ps_cols],
                                    start=(k == 0),
                                    stop=(k == len(mms) - 1),
                                )
                            osl = ot[:, mb * dma_cols + pc * ps_cols:
                                     mb * dma_cols + (pc + 1) * ps_cols]
                            if mb % 2 == 0:
                                nc.vector.tensor_copy(osl, ps[:])
                            else:
                                nc.scalar.activation(
                                    osl, ps[:],
                                    mybir.ActivationFunctionType.Copy)
                    dma_out(ot, c)
    nc.compile()
    return nc


_NC = None


def _get_nc():
    global _NC
    if _NC is None:
        _NC = build_program()
    return _NC


def pack_weights(mats):
    """(128, N_TILES*128) fp16: slot t holds permuted-BD[mb, kb] block, T."""
    bd = np.zeros((N_SAMPLES, N_SAMPLES), dtype=np.float32)
    s = 0
    for m in mats:
        g = m.shape[0]
        bd[s:s + g, s:s + g] = m
        s += g
    bdp = bd[PERM][:, PERM]
    w = np.empty((P, N_TILES * P), dtype=F16)
    for t, (mb, kb) in enumerate(PTILES):
        w[:, t * P:(t + 1) * P] = bdp[mb * P:(mb + 1) * P, kb * P:(kb + 1) * P].T
    return w


def make_in_maps(inputs, dma_cols=DMA_COLS):
    n_chunks = FREE_PER_CORE // dma_cols
    x = np.asarray(inputs["x"], dtype=np.float32)
    mats = [np.asarray(inputs[f"mat{i}"], dtype=np.float32) for i in range(15)]
    w = pack_weights(mats)
    xp = x.reshape(N_SAMPLES, FREE).astype(F16)[PERM]   # (2048, 10752)
    in_maps = []
    for c in range(N_CORES):
        xc = xp[:, c * FREE_PER_CORE:(c + 1) * FREE_PER_CORE]
        # (16 bins, 128, n_chunks, dma_cols) -> (n_chunks, 128, 16*dma_cols)
        xc = xc.reshape(NBINS, P, n_chunks, dma_cols).transpose(2, 1, 0, 3)
        in_maps.append({
            "xp": np.ascontiguousarray(xc).reshape(
                n_chunks, P, NBINS * dma_cols),
            "wpack": w,
        })
    return in_maps


def assemble(results, dma_cols=DMA_COLS):
    n_chunks = FREE_PER_CORE // dma_cols
    cols = []
    for c in range(N_CORES):
        o = results[c]["out"].reshape(n_chunks, P, NBINS, dma_cols)
        cols.append(o.transpose(2, 1, 0, 3).reshape(N_SAMPLES, FREE_PER_CORE))
    outp = np.concatenate(cols, axis=1)
    full = np.empty((N_SAMPLES, FREE), dtype=np.float32)
    full[PERM] = outp
    return full.reshape(N_SAMPLES, LENGTH, ALPHABET)


def run(inputs, nc=None, **kw):
    res = bass_utils.run_bass_kernel_spmd(
        nc if nc is not None else _get_nc(),
        make_in_maps(inputs), core_ids=list(range(N_CORES)), **kw,
    )
    return assemble(res.results), res


def kernel(**inputs):
    out, _ = run(inputs)
    return out


# revision 7
# speedup vs baseline: 2.3250x; 2.3250x over previous
"""Block-diagonal grouped matmul (nn_MatrixApply) on 8 TRN2 NeuronCores.

Math: out[s:s+g] = mat_i @ x[s:s+g] for 15 consecutive sample groups.
Equivalently out = BD @ x_flat with BD = blockdiag(mat_0..mat_14) (2048x2048)
and x_flat = x.reshape(2048, 512*21).

Sharding: sequence-parallel. The free dim L*A = 10752 is split into 8
contiguous chunks of 1344. Mats are replicated. No collectives; host
concatenates the slices.

Numerics: fp16 in / fp16 out, single matmul per tile, f32 PSUM accumulate.
Measured rel err ~5e-4 (tolerance 2e-2). This halves both input and output
HBM bytes vs the f32-equivalent and cuts PE work 3x vs bf16 hi/lo.

Layout: the 2048-sample dim is row-PERMUTED on the host into 16 bins of
exactly 128 rows (big groups split into 128-chunks; small groups/remainders
bin-packed to exact-128 bins — zero padding). x is pre-packed on the host
into the literal SBUF image (chunk, partition, bin*cols) so each chunk is
ONE fully contiguous DMA in and one out (vs 40 small strided DMAs). All
matmul tiles are full 128x128; a tile exists for each bin pair sharing a
group (36 tiles). Output is un-permuted on the host.

Per-core steady state: DMA 11MB/rep (~29us at the ~320-380GB/s/NC
effective HBM rate) overlaps PE (108 matmuls x 448 cols ~ 20us at 2.4GHz)
and PSUM evacuation split across the vector and scalar engines (~13us
each). Input DMAs ride the sync-engine HWDGE ring, output DMAs the
scalar-engine ring, so the two directions stream concurrently.
"""

import numpy as np

import concourse.bacc as bacc
import concourse.mybir as mybir
import concourse.tile as tile
from concourse import bass_utils

GROUP_SIZES = (64, 128, 256, 96, 160, 224, 192, 288, 320, 112, 80, 48, 32, 16, 32)
LENGTH = 512
ALPHABET = 21
N_SAMPLES = 2048
N_CORES = 8
FREE = LENGTH * ALPHABET            # 10752
FREE_PER_CORE = FREE // N_CORES     # 1344
P = 128
NBINS = N_SAMPLES // P              # 16
F16 = np.float16

# DMA chunking (cols per in/out DMA) and PSUM sub-chunk (cols per matmul
# accumulation pass; must divide DMA_COLS, <= 512 f32 PSUM bank).
DMA_COLS = 448
PS_COLS = 448
BUFS = 3
SPLIT_RINGS = False
ALT_RINGS = False


def _plan():
    """Row permutation into 16 exact-128 bins + matmul tile list.

    Pieces: each group is split into 128-row chunks plus a remainder;
    full chunks become bins directly, remainders/small groups are
    bin-packed (first-fit decreasing) into exact-128 bins. Returns
    (perm, ptiles): perm[i] = global sample row at packed position i;
    ptiles = [(mb, kb)] bin pairs sharing a group (full 128x128 tiles).
    """
    starts = np.cumsum((0,) + GROUP_SIZES[:-1])
    full_bins = []
    pieces = []
    for g, (gs, s0) in enumerate(zip(GROUP_SIZES, starts)):
        o = 0
        while gs - o >= P:
            full_bins.append([(g, s0 + o, P)])
            o += P
        if gs - o:
            pieces.append((gs - o, g, s0 + o))
    packed = []
    space = []
    for sz, g, s0 in sorted(pieces, reverse=True):
        for i in range(len(packed)):
            if space[i] >= sz:
                packed[i].append((g, s0, sz))
                space[i] -= sz
                break
        else:
            packed.append([(g, s0, sz)])
            space.append(P - sz)
    assert all(s == 0 for s in space), space
    bins = full_bins + packed
    assert len(bins) == NBINS
    perm = np.concatenate(
        [np.arange(s0, s0 + sz) for b in bins for (_, s0, sz) in b]
    )
    assert len(perm) == N_SAMPLES
    bin_groups = [{g for (g, _, _) in b} for b in bins]
    ptiles = [
        (mb, kb)
        for mb in range(NBINS)
        for kb in range(NBINS)
        if bin_groups[mb] & bin_groups[kb]
    ]
    return perm, ptiles


PERM, PTILES = _plan()
N_TILES = len(PTILES)               # 36
ROW_TILES = [
    [(t, kb) for t, (mb, kb) in enumerate(PTILES) if mb == i] for i in range(NBINS)
]


def build_program(reps=1, dma_cols=DMA_COLS, ps_cols=PS_COLS, bufs=BUFS,
                  split_rings=SPLIT_RINGS, alt_rings=ALT_RINGS):
    """Per-core Bass program. reps>1 repeats the streaming body in-NEFF
    for wall-clock differencing benchmarks."""
    assert FREE_PER_CORE % dma_cols == 0 and dma_cols % ps_cols == 0
    n_chunks = FREE_PER_CORE // dma_cols
    n_pass = dma_cols // ps_cols
    nc = bacc.Bacc("TRN2", target_bir_lowering=False, debug=False)
    f32 = mybir.dt.float32
    f16 = mybir.dt.float16
    xp_d = nc.dram_tensor("xp", (n_chunks, P, NBINS * dma_cols), f16,
                          kind="ExternalInput")
    w_d = nc.dram_tensor("wpack", (P, N_TILES * P), f16, kind="ExternalInput")
    op_d = nc.dram_tensor("out", (n_chunks, P, NBINS * dma_cols), f16,
                          kind="ExternalOutput")

    def dma_in(xt, c):
        if split_rings:
            h = NBINS * dma_cols // 2
            nc.sync.dma_start(xt[:, 0:h], xp_d.ap()[c, :, 0:h])
            nc.scalar.dma_start(xt[:, h:], xp_d.ap()[c, :, h:])
        else:
            eng = nc.scalar if (alt_rings and c % 2) else nc.sync
            eng.dma_start(xt[:], xp_d.ap()[c])

    def dma_out(ot, c):
        if split_rings:
            h = NBINS * dma_cols // 2
            nc.scalar.dma_start(op_d.ap()[c, :, 0:h], ot[:, 0:h])
            nc.sync.dma_start(op_d.ap()[c, :, h:], ot[:, h:])
        else:
            eng = nc.sync if (alt_rings and c % 2) else nc.scalar
            eng.dma_start(op_d.ap()[c], ot[:])

    with tile.TileContext(nc) as tc:
        with (
            tc.tile_pool(name="wpool", bufs=1) as wpool,
            tc.tile_pool(name="xpool", bufs=bufs) as xpool,
            tc.tile_pool(name="opool", bufs=bufs) as opool,
            tc.tile_pool(name="psum", bufs=8, space="PSUM") as psum_pool,
        ):
            w_sb = wpool.tile([P, N_TILES * P], f16)
            nc.scalar.dma_start(w_sb[:], w_d.ap())
            for _rep in range(reps):
                for c in range(n_chunks):
                    xt = xpool.tile([P, NBINS * dma_cols], f16, tag="x")
                    dma_in(xt, c)
                    ot = opool.tile([P, NBINS * dma_cols], f16, tag="o")
                    for pc in range(n_pass):
                        for mb in range(NBINS):
                            ps = psum_pool.tile([P, ps_cols], f32, tag="ps")
                            mms = ROW_TILES[mb]
                            for k, (t, kb) in enumerate(mms):
                                nc.tensor.matmul(
                                    ps[:],
                                    w_sb[:, t * P:(t + 1) * P],
                                    xt[:, kb * dma_cols + pc * ps_cols:
                                       kb * dma_cols + (pc + 1) * ps_cols],
                                    start=(k == 0),
                                    stop=(k == len(mms) - 1),
                                )
                            osl = ot[:, mb * dma_cols + pc * ps_cols:
                                     mb * dma_cols + (pc + 1) * ps_cols]
                            if mb % 2 == 0:
                                nc.vector.tensor_copy(osl, ps[:])
                            else:
                                nc.scalar.activation(
                                    osl, ps[:],
                                    mybir.ActivationFunctionType.Copy)
                    dma_out(ot, c)
    nc.compile()
    return nc


_NC = None


def _get_nc():
    global _NC
    if _NC is None:
        _NC = build_program()
    return _NC


def pack_weights(mats):
    """(128, N_TILES*128) fp16: slot t holds permuted-BD[mb, kb] block, T."""
    bd = np.zeros((N_SAMPLES, N_SAMPLES), dtype=np.float32)
    s = 0
    for m in mats:
        g = m.shape[0]
        bd[s:s + g, s:s + g] = m
        s += g
    bdp = bd[PERM][:, PERM]
    w = np.empty((P, N_TILES * P), dtype=F16)
    for t, (mb, kb) in enumerate(PTILES):
        w[:, t * P:(t + 1) * P] = bdp[mb * P:(mb + 1) * P, kb * P:(kb + 1) * P].T
    return w


def make_in_maps(inputs, dma_cols=DMA_COLS):
    n_chunks = FREE_PER_CORE // dma_cols
    x = np.asarray(inputs["x"], dtype=np.float32)
    mats = [np.asarray(inputs[f"mat{i}"], dtype=np.float32) for i in range(15)]
    w = pack_weights(mats)
    xp = x.reshape(N_SAMPLES, FREE).astype(F16)[PERM]   # (2048, 10752)
    in_maps = []
    for c in range(N_CORES):
        xc = xp[:, c * FREE_PER_CORE:(c + 1) * FREE_PER_CORE]
        # (16 bins, 128, n_chunks, dma_cols) -> (n_chunks, 128, 16*dma_cols)
        xc = xc.reshape(NBINS, P, n_chunks, dma_cols).transpose(2, 1, 0, 3)
        in_maps.append({
            "xp": np.ascontiguousarray(xc).reshape(
                n_chunks, P, NBINS * dma_cols),
            "wpack": w,
        })
    return in_maps


def assemble(results, dma_cols=DMA_COLS):
    n_chunks = FREE_PER_CORE // dma_cols
    cols = []
    for c in range(N_CORES):
        o = results[c]["out"].reshape(n_chunks, P, NBINS, dma_cols)
        cols.append(o.transpose(2, 1, 0, 3).reshape(N_SAMPLES, FREE_PER_CORE))
    outp = np.concatenate(cols, axis=1)
    full = np.empty((N_SAMPLES, FREE), dtype=np.float32)
    full[PERM] = outp
    return full.reshape(N_SAMPLES, LENGTH, ALPHABET)


def run(inputs, nc=None, **kw):
    res = bass_utils.run_bass_kernel_spmd(
        nc if nc is not None else _get_nc(),
        make_in_maps(inputs), core_ids=list(range(N_CORES)), **kw,
    )
    return assemble(res.results), res


def kernel(**inputs):
    out, _ = run(inputs)
    return out


# revision 8
# speedup vs baseline: 2.3951x; 1.0302x over previous
"""Block-diagonal grouped matmul (nn_MatrixApply) on 8 TRN2 NeuronCores.

Math: out[s:s+g] = mat_i @ x[s:s+g] for 15 consecutive sample groups.
Equivalently out = BD @ x_flat with BD = blockdiag(mat_0..mat_14) (2048x2048)
and x_flat = x.reshape(2048, 512*21).

Sharding: sequence-parallel. The free dim L*A = 10752 is split into 8
contiguous chunks of 1344. Mats are replicated. No collectives; host
concatenates the slices.

Numerics: fp16 in / fp16 out, single matmul per tile, f32 PSUM accumulate.
Measured rel err ~5e-4 (tolerance 2e-2). This halves both input and output
HBM bytes vs the f32-equivalent and cuts PE work 3x vs bf16 hi/lo.

Layout: the 2048-sample dim is row-PERMUTED on the host into 16 bins of
exactly 128 rows (big groups split into 128-chunks; small groups/remainders
bin-packed to exact-128 bins — zero padding). x is pre-packed on the host
into the literal SBUF image (chunk, partition, bin*cols) so each chunk is
ONE fully contiguous DMA in and one out (vs 40 small strided DMAs). All
matmul tiles are full 128x128; a tile exists for each bin pair sharing a
group (36 tiles). Output is un-permuted on the host.

Per-core steady state: DMA 11MB/rep (~29us at the ~320-380GB/s/NC
effective HBM rate) overlaps PE (108 matmuls x 448 cols ~ 20us at 2.4GHz)
and PSUM evacuation split across the vector and scalar engines (~13us
each). Input DMAs ride the sync-engine HWDGE ring, output DMAs the
scalar-engine ring, so the two directions stream concurrently.
"""

import numpy as np

import concourse.bacc as bacc
import concourse.mybir as mybir
import concourse.tile as tile
from concourse import bass_utils

GROUP_SIZES = (64, 128, 256, 96, 160, 224, 192, 288, 320, 112, 80, 48, 32, 16, 32)
LENGTH = 512
ALPHABET = 21
N_SAMPLES = 2048
N_CORES = 8
FREE = LENGTH * ALPHABET            # 10752
FREE_PER_CORE = FREE // N_CORES     # 1344
P = 128
NBINS = N_SAMPLES // P              # 16
F16 = np.float16

# DMA chunking (cols per in/out DMA) and PSUM sub-chunk (cols per matmul
# accumulation pass; must divide DMA_COLS, <= 512 f32 PSUM bank).
DMA_COLS = 448
PS_COLS = 448
BUFS = 3
SPLIT_RINGS = False
ALT_RINGS = False


def _plan():
    """Row permutation into 16 exact-128 bins + matmul tile list.

    Pieces: each group is split into 128-row chunks plus a remainder;
    full chunks become bins directly, remainders/small groups are
    bin-packed (first-fit decreasing) into exact-128 bins. Returns
    (perm, ptiles): perm[i] = global sample row at packed position i;
    ptiles = [(mb, kb)] bin pairs sharing a group (full 128x128 tiles).
    """
    starts = np.cumsum((0,) + GROUP_SIZES[:-1])
    full_bins = []
    pieces = []
    for g, (gs, s0) in enumerate(zip(GROUP_SIZES, starts)):
        o = 0
        while gs - o >= P:
            full_bins.append([(g, s0 + o, P)])
            o += P
        if gs - o:
            pieces.append((gs - o, g, s0 + o))
    packed = []
    space = []
    for sz, g, s0 in sorted(pieces, reverse=True):
        for i in range(len(packed)):
            if space[i] >= sz:
                packed[i].append((g, s0, sz))
                space[i] -= sz
                break
        else:
            packed.append([(g, s0, sz)])
            space.append(P - sz)
    assert all(s == 0 for s in space), space
    bins = full_bins + packed
    assert len(bins) == NBINS
    perm = np.concatenate(
        [np.arange(s0, s0 + sz) for b in bins for (_, s0, sz) in b]
    )
    assert len(perm) == N_SAMPLES
    bin_groups = [{g for (g, _, _) in b} for b in bins]
    ptiles = [
        (mb, kb)
        for mb in range(NBINS)
        for kb in range(NBINS)
        if bin_groups[mb] & bin_groups[kb]
    ]
    return perm, ptiles


PERM, PTILES = _plan()
N_TILES = len(PTILES)               # 36
ROW_TILES = [
    [(t, kb) for t, (mb, kb) in enumerate(PTILES) if mb == i] for i in range(NBINS)
]


def build_program(reps=1, dma_cols=DMA_COLS, ps_cols=PS_COLS, bufs=BUFS,
                  split_rings=SPLIT_RINGS, alt_rings=ALT_RINGS):
    """Per-core Bass program. reps>1 repeats the streaming body in-NEFF
    for wall-clock differencing benchmarks."""
    assert FREE_PER_CORE % dma_cols == 0 and dma_cols % ps_cols == 0
    n_chunks = FREE_PER_CORE // dma_cols
    n_pass = dma_cols // ps_cols
    nc = bacc.Bacc("TRN2", target_bir_lowering=False, debug=False)
    f32 = mybir.dt.float32
    f16 = mybir.dt.float16
    xp_d = nc.dram_tensor("xp", (n_chunks, P, NBINS * dma_cols), f16,
                          kind="ExternalInput")
    w_d = nc.dram_tensor("wpack", (P, N_TILES * P), f16, kind="ExternalInput")
    op_d = nc.dram_tensor("out", (n_chunks, P, NBINS * dma_cols), f16,
                          kind="ExternalOutput")

    def dma_in(xt, c):
        if split_rings:
            h = NBINS * dma_cols // 2
            nc.sync.dma_start(xt[:, 0:h], xp_d.ap()[c, :, 0:h])
            nc.scalar.dma_start(xt[:, h:], xp_d.ap()[c, :, h:])
        else:
            eng = nc.scalar if (alt_rings and c % 2) else nc.sync
            eng.dma_start(xt[:], xp_d.ap()[c])

    def dma_out(ot, c):
        if split_rings:
            h = NBINS * dma_cols // 2
            nc.scalar.dma_start(op_d.ap()[c, :, 0:h], ot[:, 0:h])
            nc.sync.dma_start(op_d.ap()[c, :, h:], ot[:, h:])
        else:
            eng = nc.sync if (alt_rings and c % 2) else nc.scalar
            eng.dma_start(op_d.ap()[c], ot[:])

    with tile.TileContext(nc) as tc:
        with (
            tc.tile_pool(name="wpool", bufs=1) as wpool,
            tc.tile_pool(name="xpool", bufs=bufs) as xpool,
            tc.tile_pool(name="opool", bufs=bufs) as opool,
            tc.tile_pool(name="psum", bufs=8, space="PSUM") as psum_pool,
        ):
            w_sb = wpool.tile([P, N_TILES * P], f16)
            nc.scalar.dma_start(w_sb[:], w_d.ap())
            for _rep in range(reps):
                for c in range(n_chunks):
                    xt = xpool.tile([P, NBINS * dma_cols], f16, tag="x")
                    dma_in(xt, c)
                    ot = opool.tile([P, NBINS * dma_cols], f16, tag="o")
                    for pc in range(n_pass):
                        for mb in range(NBINS):
                            ps = psum_pool.tile([P, ps_cols], f32, tag="ps")
                            mms = ROW_TILES[mb]
                            for k, (t, kb) in enumerate(mms):
                                nc.tensor.matmul(
                                    ps[:],
                                    w_sb[:, t * P:(t + 1) * P],
                                    xt[:, kb * dma_cols + pc * ps_cols:
                                       kb * dma_cols + (pc + 1) * ps_cols],
                                    start=(k == 0),
                                    stop=(k == len(mms) - 1),
                                )
                            osl = ot[:, mb * dma_cols + pc * ps_cols:
                                     mb * dma_cols + (pc + 1) * ps_cols]
                            # 2:1 DVE:ACT split — DVE cost is well-modeled
                            # (~0.53us/copy); ACT carries the remainder so
                            # neither engine approaches the ~29us DMA floor
                            # even if the ScalarE slow-copy errata applies.
                            if mb % 3 != 2:
                                nc.vector.tensor_copy(osl, ps[:])
                            else:
                                nc.scalar.activation(
                                    osl, ps[:],
                                    mybir.ActivationFunctionType.Copy)
                    dma_out(ot, c)
    nc.compile()
    return nc


_NC = None


def _get_nc():
    global _NC
    if _NC is None:
        _NC = build_program()
    return _NC


def pack_weights(mats):
    """(128, N_TILES*128) fp16: slot t holds permuted-BD[mb, kb] block, T."""
    bd = np.zeros((N_SAMPLES, N_SAMPLES), dtype=np.float32)
    s = 0
    for m in mats:
        g = m.shape[0]
        bd[s:s + g, s:s + g] = m
        s += g
    bdp = bd[PERM][:, PERM]
    w = np.empty((P, N_TILES * P), dtype=F16)
    for t, (mb, kb) in enumerate(PTILES):
        w[:, t * P:(t + 1) * P] = bdp[mb * P:(mb + 1) * P, kb * P:(kb + 1) * P].T
    return w


def make_in_maps(inputs, dma_cols=DMA_COLS):
    n_chunks = FREE_PER_CORE // dma_cols
    x = np.asarray(inputs["x"], dtype=np.float32)
    mats = [np.asarray(inputs[f"mat{i}"], dtype=np.float32) for i in range(15)]
    w = pack_weights(mats)
    xp = x.reshape(N_SAMPLES, FREE).astype(F16)[PERM]   # (2048, 10752)
    in_maps = []
    for c in range(N_CORES):
        xc = xp[:, c * FREE_PER_CORE:(c + 1) * FREE_PER_CORE]
        # (16 bins, 128, n_chunks, dma_cols) -> (n_chunks, 128, 16*dma_cols)
        xc = xc.reshape(NBINS, P, n_chunks, dma_cols).transpose(2, 1, 0, 3)
        in_maps.append({
            "xp": np.ascontiguousarray(xc).reshape(
                n_chunks, P, NBINS * dma_cols),
            "wpack": w,
        })
    return in_maps


def assemble(results, dma_cols=DMA_COLS):
    n_chunks = FREE_PER_CORE // dma_cols
    cols = []
    for c in range(N_CORES):
        o = results[c]["out"].reshape(n_chunks, P, NBINS, dma_cols)
        cols.append(o.transpose(2, 1, 0, 3).reshape(N_SAMPLES, FREE_PER_CORE))
    outp = np.concatenate(cols, axis=1)
    full = np.empty((N_SAMPLES, FREE), dtype=np.float32)
    full[PERM] = outp
    return full.reshape(N_SAMPLES, LENGTH, ALPHABET)


def run(inputs, nc=None, **kw):
    res = bass_utils.run_bass_kernel_spmd(
        nc if nc is not None else _get_nc(),
        make_in_maps(inputs), core_ids=list(range(N_CORES)), **kw,
    )
    return assemble(res.results), res


def kernel(**inputs):
    out, _ = run(inputs)
    return out
